# revision 17
# baseline (speedup 1.0000x reference)
"""Trainium2 Bass kernel for nn_DKT_14534169330363 (liquid-time-constant DKT).

Math (reference):
    idx  = q + 1024*r                       [B=64, S=512]
    xemb = emb[idx]                         [B, S, E=512]
    xp   = xemb @ Wx^T + Wx_b + Wh_b        [B, S, H=512]
    h_t  = h_{t-1} + (-h_{t-1} + tanh(h_{t-1} @ Wh^T + xp_t)) / tau
    y    = sigmoid(hs @ Wo^T + Wo_b)        [B, S, C=1024]

Sharding: data-parallel over batch, 8 cores x 8 rows each; weights replicated.

Per-core device schedule (all matmul operands bf16, f32 accumulation):
  A. DMA pre-transposed/cast weights into SBUF.
  B. emb2 = emb @ Wx^T + (Wx_b + Wh_b)  -> DRAM scratch [2048, 512] bf16
     (gather table: xp rows become emb2[idx] -- the xproj GEMM collapses
     into the embedding gather).
  C. dma_gather pulls emb2 rows for this core's tokens (s-major order,
     tokens on partitions); PE transposes 128x128 blocks into xpT
     [128, group, chunk, token] (hidden on partitions).
  D. 512-step recurrence in hT layout (hidden on partitions):
       z[:, (j,b)] = I.T @ xpT_t  (one matmul seeds PSUM with xp)
                   + sum_k WhT[k,j].T @ hT_{t-1}[k]   (16 matmuls)
       hT_t = tanh(z)  (2 ACT ops of [128,16], cast to bf16 into hsT)
  E. Output projection: y-tile[tok, c] = sigmoid(hsT-tiles.T @ WoT + b_o),
     DMA to DRAM with (s, b)-interleaved access pattern.
"""

import sys

for _p in ("/opt/trn_rl_repo", "/root/.axon_site/_ro/trn_rl_repo"):
    if _p not in sys.path:
        sys.path.append(_p)

import numpy as np
import ml_dtypes

import concourse.bass as bass
import concourse.mybir as mybir
import concourse.tile as tile
from concourse import bacc
from concourse.bass_utils import run_bass_kernel_spmd

BF16 = ml_dtypes.bfloat16

NUM_C = 1024
EMB = 512
HID = 512
BATCH = 64
SEQ = 512
N_CORES = 8
B_LOC = BATCH // N_CORES          # 8 batch rows per core
NROW = 2 * NUM_C                  # 2048 emb table rows
KC = HID // 128                   # 4 hidden chunks
GATHER_STEPS = 64                 # steps per gather chunk
F32 = mybir.dt.float32


def build_nc(S=SEQ, general_tau=False, stop_stage="E", dbg=""):
    """Build the per-core Bass program (same NEFF for all cores, SPMD)."""
    dt = mybir.dt
    nc = bacc.Bacc(None, target_bir_lowering=False)
    stages = {"A": 0, "B": 1, "C": 2, "D": 3, "E": 4}[stop_stage]
    no_wh = "nowh" in dbg      # timing-only: skip Wh matmuls
    no_act = "noact" in dbg    # timing-only: skip tanh ACTs

    NG = S // GATHER_STEPS                  # number of gather chunks
    NTOK = S * B_LOC                        # tokens per core
    NTT = NTOK // 128                       # 128-token output tiles

    # ---- DRAM I/O ----
    embT_d = nc.dram_tensor("embT", [EMB, NROW], dt.bfloat16, kind="ExternalInput")
    wxT_d = nc.dram_tensor("wxT", [EMB, HID], dt.bfloat16, kind="ExternalInput")
    whT_d = nc.dram_tensor("whT", [HID, HID], dt.bfloat16, kind="ExternalInput")
    woT_d = nc.dram_tensor("woT", [HID, NUM_C], dt.bfloat16, kind="ExternalInput")
    bx_d = nc.dram_tensor("biasx", [1, HID], dt.bfloat16, kind="ExternalInput")
    bo_d = nc.dram_tensor("biaso", [1, NUM_C], dt.bfloat16, kind="ExternalInput")
    eye_d = nc.dram_tensor("eye", [128, 128], dt.bfloat16, kind="ExternalInput")
    ones_d = nc.dram_tensor("ones", [1, 128], dt.bfloat16, kind="ExternalInput")
    idx_d = nc.dram_tensor("idxs", [128, NTOK // 16], dt.int16, kind="ExternalInput")
    if general_tau:
        ta_d = nc.dram_tensor("taua", [128, 32], dt.float32, kind="ExternalInput")
        tb_d = nc.dram_tensor("taub", [128, 32], dt.float32, kind="ExternalInput")
    y_d = nc.dram_tensor("y", [B_LOC, S, NUM_C], dt.float32, kind="ExternalOutput")

    emb2_d = nc.dram_tensor("emb2", [NROW, HID], dt.bfloat16)  # Internal scratch

    with tile.TileContext(nc) as tc:
        with (
            tc.tile_pool(name="weights", bufs=1) as wpool,
            tc.tile_pool(name="state", bufs=1) as spool,
            tc.tile_pool(name="e2sb", bufs=3) as e2pool,
            tc.tile_pool(name="ysb", bufs=3) as ypool,
            tc.tile_pool(name="zps", bufs=6, space="PSUM") as zpool,
            tc.tile_pool(name="gps", bufs=2, space="PSUM") as gpool,
        ):
            # ---- stage A: load weights ----
            embT = wpool.tile([128, KC, NROW], dt.bfloat16)
            wxT = wpool.tile([128, KC, HID], dt.bfloat16)
            whT = wpool.tile([128, KC, HID], dt.bfloat16)
            woT = wpool.tile([128, KC, NUM_C], dt.bfloat16)
            bx = wpool.tile([1, HID], dt.bfloat16)
            bo = wpool.tile([1, NUM_C], dt.bfloat16)
            eye = wpool.tile([128, 128], dt.bfloat16)
            ones = wpool.tile([1, 128], dt.bfloat16)
            idxs = wpool.tile([128, NTOK // 16], dt.int16)
            for k in range(KC):
                nc.sync.dma_start(embT[:, k, :], embT_d[128 * k:128 * (k + 1), :])
                nc.sync.dma_start(wxT[:, k, :], wxT_d[128 * k:128 * (k + 1), :])
                nc.sync.dma_start(whT[:, k, :], whT_d[128 * k:128 * (k + 1), :])
                nc.sync.dma_start(woT[:, k, :], woT_d[128 * k:128 * (k + 1), :])
            nc.sync.dma_start(bx[:], bx_d[:, :])
            nc.sync.dma_start(bo[:], bo_d[:, :])
            nc.sync.dma_start(eye[:], eye_d[:, :])
            nc.sync.dma_start(ones[:], ones_d[:, :])
            nc.sync.dma_start(idxs[:], idx_d[:, :])
            if general_tau:
                taua = wpool.tile([128, 32], dt.float32)
                taub = wpool.tile([128, 32], dt.float32)
                nc.sync.dma_start(taua[:], ta_d[:, :])
                nc.sync.dma_start(taub[:], tb_d[:, :])

            # ---- stage B: emb2 = emb @ Wx^T + bias -> DRAM ----
            for rt in range(NROW // 128 if stages >= 1 else 0):
                ps = gpool.tile([128, HID], dt.float32, tag="gemmps")
                for k in range(KC):
                    nc.tensor.matmul(
                        ps[:, :],
                        embT[:, k, 128 * rt:128 * (rt + 1)],
                        wxT[:, k, :],
                        start=(k == 0), stop=False,
                    )
                nc.tensor.matmul(ps[:, :], ones[:, :], bx[:, :],
                                 start=False, stop=True)
                e2 = e2pool.tile([128, HID], dt.bfloat16, tag="e2")
                nc.vector.tensor_copy(e2[:, :], ps[:, :])
                nc.sync.dma_start(emb2_d[128 * rt:128 * (rt + 1), :], e2[:, :])

            # ---- stage C: plain gathers (tokens on partitions), then
            # PE-transpose 128x128 blocks into xpT [128, NG, KC, 512] ----
            xpT = spool.tile([128, NG, KC, 512], dt.bfloat16)
            for g in range(NG if stages >= 2 else 0):
                xg = e2pool.tile([128, KC, HID], dt.bfloat16, tag="xg")
                nc.gpsimd.dma_gather(
                    out_ap=xg[:, :, :],
                    in_ap=emb2_d[:, :],
                    idxs_ap=idxs[:, 32 * g:32 * (g + 1)],
                    num_idxs=512,
                    num_idxs_reg=512,
                    elem_size=HID,
                )
                for tg in range(4):          # token sub-tile within group
                    for ec in range(KC):     # hidden chunk
                        pt = gpool.tile([128, 128], dt.bfloat16, tag="gemmps")
                        nc.tensor.transpose(
                            pt[:, :], xg[:, tg, 128 * ec:128 * (ec + 1)],
                            eye[:, :])
                        nc.vector.tensor_copy(
                            xpT[:, g, ec, 128 * tg:128 * (tg + 1)], pt[:, :])

            # ---- stage D: recurrence ----
            hsT = spool.tile([128, KC, NTOK], dt.bfloat16)
            Tanh = mybir.ActivationFunctionType.Tanh
            fast2 = not general_tau and not no_wh and not no_act \
                and "v1" not in dbg and "act1" not in dbg \
                and "wh8" not in dbg and "jmaj" not in dbg
            for t in range(S if stages >= 3 else 0):
                g, o = t // GATHER_STEPS, t % GATHER_STEPS
                if fast2:
                    # two banks per step: tanh(j01) overlaps j23 matmuls
                    prev = slice(B_LOC * (t - 1), B_LOC * t)
                    Tslc = slice(B_LOC * o, B_LOC * (o + 1))
                    for jh in range(2):
                        zt = zpool.tile([128, 512], dt.float32, tag="z")
                        nc.tensor.matmul(
                            zt[:, 0:2 * B_LOC], eye[:, :],
                            xpT[:, g, 2 * jh:2 * jh + 2, Tslc],
                            start=True, stop=(t == 0),
                        )
                        if t > 0:
                            for kh in range(2):
                                for jj in range(2):
                                    j = 2 * jh + jj
                                    for k in (2 * kh, 2 * kh + 1):
                                        nc.tensor.matmul(
                                            zt[:, B_LOC * jj:B_LOC * (jj + 1)],
                                            whT[:, k, 128 * j:128 * (j + 1)],
                                            hsT[:, k, prev],
                                            start=False,
                                            stop=(kh == 1 and jj == 1 and k == 2 * kh + 1),
                                        )
                        nc.scalar.activation(
                            hsT[:, 2 * jh:2 * jh + 2, B_LOC * t:B_LOC * (t + 1)],
                            zt[:, 0:2 * B_LOC],
                            Tanh,
                        )
                    continue
                zfull = zpool.tile([128, 512], dt.float32, tag="z")  # full bank
                z = zfull[:, 0:4 * B_LOC]
                nc.tensor.matmul(
                    z, eye[:, :],
                    xpT[:, g, :, B_LOC * o:B_LOC * (o + 1)],
                    start=True, stop=(t == 0 or no_wh),
                )
                if t > 0 and not no_wh:
                    prev = slice(B_LOC * (t - 1), B_LOC * t)
                    if "jmaj" in dbg:
                        for j in range(KC):
                            for k in range(KC):
                                nc.tensor.matmul(
                                    zfull[:, B_LOC * j:B_LOC * (j + 1)],
                                    whT[:, k, 128 * j:128 * (j + 1)],
                                    hsT[:, k, prev],
                                    start=False,
                                    stop=(j == KC - 1 and k == KC - 1),
                                )
                    else:
                        for khalf in range(1 if "wh8" in dbg else 2):
                            for j in range(KC):
                                for k in (2 * khalf, 2 * khalf + 1):
                                    nc.tensor.matmul(
                                        zfull[:, B_LOC * j:B_LOC * (j + 1)],
                                        whT[:, k, 128 * j:128 * (j + 1)],
                                        hsT[:, k, prev],
                                        start=False,
                                        stop=(khalf == (0 if "wh8" in dbg else 1)
                                              and j == KC - 1 and k % 2 == 1),
                                    )
                if no_act:
                    pass
                elif "act1" in dbg and not general_tau:
                    nc.scalar.activation(
                        hsT[:, :, B_LOC * t:B_LOC * (t + 1)], z[:, :], Tanh)
                elif not general_tau:
                    for hf in range(2):
                        nc.scalar.activation(
                            hsT[:, 2 * hf:2 * hf + 2, B_LOC * t:B_LOC * (t + 1)],
                            z[:, 16 * hf:16 * (hf + 1)],
                            Tanh,
                        )
                else:
                    th = spool.tile([128, 32], dt.float32, tag="th")
                    nc.scalar.activation(th[:, :], z[:, :], Tanh)
                    cur = hsT[:, :, B_LOC * t:B_LOC * (t + 1)]
                    if t == 0:
                        nc.vector.tensor_mul(cur, th[:, :], taub[:, :])
                    else:
                        ha = spool.tile([128, 32], dt.float32, tag="ha")
                        nc.vector.tensor_mul(
                            ha[:, :], hsT[:, :, prev], taua[:, :])
                        nc.vector.tensor_mul(th[:, :], th[:, :], taub[:, :])
                        nc.vector.tensor_add(cur, ha[:, :], th[:, :])

            # ---- stage E: output projection + sigmoid + store ----
            Sig = mybir.ActivationFunctionType.Sigmoid
            for tt in range(NTT if stages >= 4 else 0):
                for ch in range(NUM_C // 512):
                    ps = gpool.tile([128, 512], dt.float32, tag="gemmps")
                    for k in range(KC):
                        nc.tensor.matmul(
                            ps[:, :],
                            hsT[:, k, 128 * tt:128 * (tt + 1)],
                            woT[:, k, 512 * ch:512 * (ch + 1)],
                            start=(k == 0), stop=False,
                        )
                    nc.tensor.matmul(ps[:, :], ones[:, :],
                                     bo[:, 512 * ch:512 * (ch + 1)],
                                     start=False, stop=True)
                    ysb = ypool.tile([128, 512], dt.float32, tag="y")
                    nc.scalar.activation(ysb[:, :], ps[:, :], Sig)
                    # tokens are s-major: token = s*B_LOC + b -> y[b, s, c]
                    out_ap = bass.AP(
                        y_d, (16 * tt) * NUM_C + 512 * ch,
                        [[NUM_C, 16], [S * NUM_C, B_LOC], [1, 512]],
                    )
                    nc.sync.dma_start(out_ap, ysb[:, :])

    nc.compile()
    return nc


def _host_prep(q, r, emb, Wh_w, Wh_b, Wx_w, Wx_b, tau, Wo_w, Wo_b, S=SEQ):
    """Host-side layout prep: transpose/cast weights, build per-core index maps."""
    general_tau = not np.allclose(np.asarray(tau, np.float32), 1.0)
    common = {
        "embT": np.ascontiguousarray(np.asarray(emb, np.float32).T).astype(BF16),
        "wxT": np.ascontiguousarray(np.asarray(Wx_w, np.float32).T).astype(BF16),
        "whT": np.ascontiguousarray(np.asarray(Wh_w, np.float32).T).astype(BF16),
        "woT": np.ascontiguousarray(np.asarray(Wo_w, np.float32).T).astype(BF16),
        "biasx": (np.asarray(Wx_b, np.float32)
                  + np.asarray(Wh_b, np.float32)).reshape(1, HID).astype(BF16),
        "biaso": np.asarray(Wo_b, np.float32).reshape(1, NUM_C).astype(BF16),
        "eye": np.eye(128, dtype=np.float32).astype(BF16),
        "ones": np.ones((1, 128), np.float32).astype(BF16),
    }
    if general_tau:
        inv = (1.0 / np.asarray(tau, np.float32)).astype(np.float32)
        a = (1.0 - inv).reshape(KC, 128).T            # [128, KC]
        b = inv.reshape(KC, 128).T
        common["taua"] = np.repeat(a, B_LOC, axis=1).astype(np.float32)
        common["taub"] = np.repeat(b, B_LOC, axis=1).astype(np.float32)

    idx_full = (np.asarray(q, np.int64) + NUM_C * np.asarray(r, np.int64))
    idx_full = idx_full.astype(np.int16)              # values < 2048
    in_maps = []
    for c in range(N_CORES):
        shard = idx_full[c * B_LOC:(c + 1) * B_LOC, :S]     # [B_LOC, S]
        idx_sb = np.ascontiguousarray(shard.T).reshape(-1)  # s-major tokens
        ntok = idx_sb.size
        wrap = idx_sb.reshape(ntok // 512, 32, 16).transpose(2, 0, 1)
        wrap = np.ascontiguousarray(wrap).reshape(16, ntok // 16)
        idxs = np.tile(wrap, (8, 1)).astype(np.int16)       # [128, ntok//16]
        in_maps.append({**common, "idxs": idxs})
    return in_maps, general_tau


_NC_CACHE = {}


def _get_nc(S, general_tau):
    key = (S, general_tau)
    if key not in _NC_CACHE:
        _NC_CACHE[key] = build_nc(S=S, general_tau=general_tau)
    return _NC_CACHE[key]


def run(trace=False, S=SEQ, **inputs):
    in_maps, general_tau = _host_prep(S=S, **inputs)
    nc = _get_nc(S, general_tau)
    last_err = None
    for _attempt in range(3):   # NRT exec errors are occasionally transient
        try:
            res = run_bass_kernel_spmd(nc, in_maps,
                                       core_ids=list(range(N_CORES)),
                                       trace=trace)
            break
        except Exception as e:  # noqa: BLE001
            last_err = e
    else:
        raise last_err
    y = np.concatenate([r["y"] for r in res.results], axis=0)
    return y.astype(np.float32), res


def kernel(**inputs) -> np.ndarray:
    y, _ = run(trace=False, **inputs)
    return y


# revision 18
# speedup vs baseline: 1.1874x; 1.1874x over previous
"""Trainium2 Bass kernel for nn_DKT_14534169330363 (liquid-time-constant DKT).

Math (reference):
    idx  = q + 1024*r                       [B=64, S=512]
    xemb = emb[idx]                         [B, S, E=512]
    xp   = xemb @ Wx^T + Wx_b + Wh_b        [B, S, H=512]
    h_t  = h_{t-1} + (-h_{t-1} + tanh(h_{t-1} @ Wh^T + xp_t)) / tau
    y    = sigmoid(hs @ Wo^T + Wo_b)        [B, S, C=1024]

Sharding: data-parallel over batch, 8 cores x 8 rows each; weights replicated.

Per-core device schedule (all matmul operands bf16, f32 accumulation):
  A. DMA pre-transposed/cast weights into SBUF.
  B. emb2 = emb @ Wx^T + (Wx_b + Wh_b)  -> DRAM scratch [2048, 512] bf16
     (gather table: xp rows become emb2[idx] -- the xproj GEMM collapses
     into the embedding gather).
  C. dma_gather pulls emb2 rows for this core's tokens (s-major order,
     tokens on partitions); PE transposes 128x128 blocks into xpT
     [128, group, chunk, token] (hidden on partitions).
  D. 512-step recurrence in hT layout (hidden on partitions):
       z[:, (j,b)] = I.T @ xpT_t  (one matmul seeds PSUM with xp)
                   + sum_k WhT[k,j].T @ hT_{t-1}[k]   (16 matmuls)
       hT_t = tanh(z)  (2 ACT ops of [128,16], cast to bf16 into hsT)
  E. Output projection: y-tile[tok, c] = sigmoid(hsT-tiles.T @ WoT + b_o),
     DMA to DRAM with (s, b)-interleaved access pattern.
"""

import sys

for _p in ("/opt/trn_rl_repo", "/root/.axon_site/_ro/trn_rl_repo"):
    if _p not in sys.path:
        sys.path.append(_p)

import numpy as np
import ml_dtypes

import concourse.bass as bass
import concourse.mybir as mybir
import concourse.tile as tile
from concourse import bacc
from concourse.bass_utils import run_bass_kernel_spmd

BF16 = ml_dtypes.bfloat16

NUM_C = 1024
EMB = 512
HID = 512
BATCH = 64
SEQ = 512
N_CORES = 8
B_LOC = BATCH // N_CORES          # 8 batch rows per core
NROW = 2 * NUM_C                  # 2048 emb table rows
KC = HID // 128                   # 4 hidden chunks
GATHER_STEPS = 64                 # steps per gather chunk
F32 = mybir.dt.float32


def build_nc(S=SEQ, general_tau=False, stop_stage="E", dbg=""):
    """Build the per-core Bass program (same NEFF for all cores, SPMD)."""
    dt = mybir.dt
    nc = bacc.Bacc(None, target_bir_lowering=False)
    stages = {"A": 0, "B": 1, "C": 2, "D": 3, "E": 4}[stop_stage]
    no_wh = "nowh" in dbg      # timing-only: skip Wh matmuls
    no_act = "noact" in dbg    # timing-only: skip tanh ACTs

    NG = S // GATHER_STEPS                  # number of gather chunks
    NTOK = S * B_LOC                        # tokens per core
    NTT = NTOK // 128                       # 128-token output tiles

    # ---- DRAM I/O ----
    embT_d = nc.dram_tensor("embT", [EMB, NROW], dt.bfloat16, kind="ExternalInput")
    wxT_d = nc.dram_tensor("wxT", [EMB, HID], dt.bfloat16, kind="ExternalInput")
    whT_d = nc.dram_tensor("whT", [HID, HID], dt.bfloat16, kind="ExternalInput")
    woT_d = nc.dram_tensor("woT", [HID, NUM_C], dt.bfloat16, kind="ExternalInput")
    bx_d = nc.dram_tensor("biasx", [1, HID], dt.bfloat16, kind="ExternalInput")
    bo_d = nc.dram_tensor("biaso", [1, NUM_C], dt.bfloat16, kind="ExternalInput")
    eye_d = nc.dram_tensor("eye", [128, 128], dt.bfloat16, kind="ExternalInput")
    ones_d = nc.dram_tensor("ones", [1, 128], dt.bfloat16, kind="ExternalInput")
    idx_d = nc.dram_tensor("idxs", [128, NTOK // 16], dt.int16, kind="ExternalInput")
    if general_tau:
        ta_d = nc.dram_tensor("taua", [128, 32], dt.float32, kind="ExternalInput")
        tb_d = nc.dram_tensor("taub", [128, 32], dt.float32, kind="ExternalInput")
    y_d = nc.dram_tensor("y", [B_LOC, S, NUM_C], dt.float32, kind="ExternalOutput")

    emb2_d = nc.dram_tensor("emb2", [NROW, HID], dt.bfloat16)  # Internal scratch

    with tile.TileContext(nc) as tc:
        with (
            tc.tile_pool(name="weights", bufs=1) as wpool,
            tc.tile_pool(name="state", bufs=1) as spool,
            tc.tile_pool(name="e2sb", bufs=3) as e2pool,
            tc.tile_pool(name="ysb", bufs=3) as ypool,
            tc.tile_pool(name="zps", bufs=6, space="PSUM") as zpool,
            tc.tile_pool(name="gps", bufs=2, space="PSUM") as gpool,
        ):
            # ---- stage A: load weights ----
            embT = wpool.tile([128, KC, NROW], dt.bfloat16)
            wxT = wpool.tile([128, KC, HID], dt.bfloat16)
            whT = wpool.tile([128, KC, HID], dt.bfloat16)
            woT = wpool.tile([128, KC, NUM_C], dt.bfloat16)
            bx = wpool.tile([1, HID], dt.bfloat16)
            bo = wpool.tile([1, NUM_C], dt.bfloat16)
            eye = wpool.tile([128, 128], dt.bfloat16)
            ones = wpool.tile([1, 128], dt.bfloat16)
            idxs = wpool.tile([128, NTOK // 16], dt.int16)
            for k in range(KC):
                nc.sync.dma_start(embT[:, k, :], embT_d[128 * k:128 * (k + 1), :])
                nc.sync.dma_start(wxT[:, k, :], wxT_d[128 * k:128 * (k + 1), :])
                nc.sync.dma_start(whT[:, k, :], whT_d[128 * k:128 * (k + 1), :])
                nc.sync.dma_start(woT[:, k, :], woT_d[128 * k:128 * (k + 1), :])
            nc.sync.dma_start(bx[:], bx_d[:, :])
            nc.sync.dma_start(bo[:], bo_d[:, :])
            nc.sync.dma_start(eye[:], eye_d[:, :])
            nc.sync.dma_start(ones[:], ones_d[:, :])
            nc.sync.dma_start(idxs[:], idx_d[:, :])
            if general_tau:
                taua = wpool.tile([128, 32], dt.float32)
                taub = wpool.tile([128, 32], dt.float32)
                nc.sync.dma_start(taua[:], ta_d[:, :])
                nc.sync.dma_start(taub[:], tb_d[:, :])

            # ---- stage B: emb2 = emb @ Wx^T + bias -> DRAM ----
            for rt in range(NROW // 128 if stages >= 1 else 0):
                ps = gpool.tile([128, HID], dt.float32, tag="gemmps")
                for k in range(KC):
                    nc.tensor.matmul(
                        ps[:, :],
                        embT[:, k, 128 * rt:128 * (rt + 1)],
                        wxT[:, k, :],
                        start=(k == 0), stop=False,
                    )
                nc.tensor.matmul(ps[:, :], ones[:, :], bx[:, :],
                                 start=False, stop=True)
                e2 = e2pool.tile([128, HID], dt.bfloat16, tag="e2")
                nc.vector.tensor_copy(e2[:, :], ps[:, :])
                nc.sync.dma_start(emb2_d[128 * rt:128 * (rt + 1), :], e2[:, :])

            # ---- stage C: plain gathers (tokens on partitions), then
            # PE-transpose 128x128 blocks into xpT [128, NG, KC, 512] ----
            xpT = spool.tile([128, NG, KC, 512], dt.bfloat16)
            for g in range(NG if stages >= 2 else 0):
                xg = e2pool.tile([128, KC, HID], dt.bfloat16, tag="xg")
                nc.gpsimd.dma_gather(
                    out_ap=xg[:, :, :],
                    in_ap=emb2_d[:, :],
                    idxs_ap=idxs[:, 32 * g:32 * (g + 1)],
                    num_idxs=512,
                    num_idxs_reg=512,
                    elem_size=HID,
                )
                for tg in range(4):          # token sub-tile within group
                    for ec in range(KC):     # hidden chunk
                        pt = gpool.tile([128, 128], dt.bfloat16, tag="gemmps")
                        nc.tensor.transpose(
                            pt[:, :], xg[:, tg, 128 * ec:128 * (ec + 1)],
                            eye[:, :])
                        nc.vector.tensor_copy(
                            xpT[:, g, ec, 128 * tg:128 * (tg + 1)], pt[:, :])

            # ---- stage D: recurrence ----
            hsT = spool.tile([128, KC, NTOK], dt.bfloat16)
            Tanh = mybir.ActivationFunctionType.Tanh
            # A/B-tested 2026-08-04: the single-bank k-half-split step (v1)
            # beat the two-bank j-half split (7/9 paired reps, ~0.7us/step);
            # the split halves PSUM reuse distance and adds an xp matmul.
            fast2 = "v2" in dbg and not general_tau and not no_wh and not no_act
            for t in range(S if stages >= 3 else 0):
                g, o = t // GATHER_STEPS, t % GATHER_STEPS
                if fast2:
                    # two banks per step: tanh(j01) overlaps j23 matmuls
                    prev = slice(B_LOC * (t - 1), B_LOC * t)
                    Tslc = slice(B_LOC * o, B_LOC * (o + 1))
                    for jh in range(2):
                        zt = zpool.tile([128, 512], dt.float32, tag="z")
                        nc.tensor.matmul(
                            zt[:, 0:2 * B_LOC], eye[:, :],
                            xpT[:, g, 2 * jh:2 * jh + 2, Tslc],
                            start=True, stop=(t == 0),
                        )
                        if t > 0:
                            for kh in range(2):
                                for jj in range(2):
                                    j = 2 * jh + jj
                                    for k in (2 * kh, 2 * kh + 1):
                                        nc.tensor.matmul(
                                            zt[:, B_LOC * jj:B_LOC * (jj + 1)],
                                            whT[:, k, 128 * j:128 * (j + 1)],
                                            hsT[:, k, prev],
                                            start=False,
                                            stop=(kh == 1 and jj == 1 and k == 2 * kh + 1),
                                        )
                        nc.scalar.activation(
                            hsT[:, 2 * jh:2 * jh + 2, B_LOC * t:B_LOC * (t + 1)],
                            zt[:, 0:2 * B_LOC],
                            Tanh,
                        )
                    continue
                zfull = zpool.tile([128, 512], dt.float32, tag="z")  # full bank
                z = zfull[:, 0:4 * B_LOC]
                nc.tensor.matmul(
                    z, eye[:, :],
                    xpT[:, g, :, B_LOC * o:B_LOC * (o + 1)],
                    start=True, stop=(t == 0 or no_wh),
                )
                if t > 0 and not no_wh:
                    prev = slice(B_LOC * (t - 1), B_LOC * t)
                    if "jmaj" in dbg:
                        for j in range(KC):
                            for k in range(KC):
                                nc.tensor.matmul(
                                    zfull[:, B_LOC * j:B_LOC * (j + 1)],
                                    whT[:, k, 128 * j:128 * (j + 1)],
                                    hsT[:, k, prev],
                                    start=False,
                                    stop=(j == KC - 1 and k == KC - 1),
                                )
                    else:
                        for khalf in range(1 if "wh8" in dbg else 2):
                            for j in range(KC):
                                for k in (2 * khalf, 2 * khalf + 1):
                                    nc.tensor.matmul(
                                        zfull[:, B_LOC * j:B_LOC * (j + 1)],
                                        whT[:, k, 128 * j:128 * (j + 1)],
                                        hsT[:, k, prev],
                                        start=False,
                                        stop=(khalf == (0 if "wh8" in dbg else 1)
                                              and j == KC - 1 and k % 2 == 1),
                                    )
                if no_act:
                    pass
                elif "act1" in dbg and not general_tau:
                    nc.scalar.activation(
                        hsT[:, :, B_LOC * t:B_LOC * (t + 1)], z[:, :], Tanh)
                elif not general_tau:
                    for hf in range(2):
                        nc.scalar.activation(
                            hsT[:, 2 * hf:2 * hf + 2, B_LOC * t:B_LOC * (t + 1)],
                            z[:, 16 * hf:16 * (hf + 1)],
                            Tanh,
                        )
                else:
                    th = spool.tile([128, 32], dt.float32, tag="th")
                    nc.scalar.activation(th[:, :], z[:, :], Tanh)
                    cur = hsT[:, :, B_LOC * t:B_LOC * (t + 1)]
                    if t == 0:
                        nc.vector.tensor_mul(cur, th[:, :], taub[:, :])
                    else:
                        ha = spool.tile([128, 32], dt.float32, tag="ha")
                        nc.vector.tensor_mul(
                            ha[:, :], hsT[:, :, prev], taua[:, :])
                        nc.vector.tensor_mul(th[:, :], th[:, :], taub[:, :])
                        nc.vector.tensor_add(cur, ha[:, :], th[:, :])

            # ---- stage E: output projection + sigmoid + store ----
            Sig = mybir.ActivationFunctionType.Sigmoid
            for tt in range(NTT if stages >= 4 else 0):
                for ch in range(NUM_C // 512):
                    ps = gpool.tile([128, 512], dt.float32, tag="gemmps")
                    for k in range(KC):
                        nc.tensor.matmul(
                            ps[:, :],
                            hsT[:, k, 128 * tt:128 * (tt + 1)],
                            woT[:, k, 512 * ch:512 * (ch + 1)],
                            start=(k == 0), stop=False,
                        )
                    nc.tensor.matmul(ps[:, :], ones[:, :],
                                     bo[:, 512 * ch:512 * (ch + 1)],
                                     start=False, stop=True)
                    ysb = ypool.tile([128, 512], dt.float32, tag="y")
                    nc.scalar.activation(ysb[:, :], ps[:, :], Sig)
                    # tokens are s-major: token = s*B_LOC + b -> y[b, s, c]
                    out_ap = bass.AP(
                        y_d, (16 * tt) * NUM_C + 512 * ch,
                        [[NUM_C, 16], [S * NUM_C, B_LOC], [1, 512]],
                    )
                    nc.sync.dma_start(out_ap, ysb[:, :])

    nc.compile()
    return nc


def _host_prep(q, r, emb, Wh_w, Wh_b, Wx_w, Wx_b, tau, Wo_w, Wo_b, S=SEQ):
    """Host-side layout prep: transpose/cast weights, build per-core index maps."""
    general_tau = not np.allclose(np.asarray(tau, np.float32), 1.0)
    common = {
        "embT": np.ascontiguousarray(np.asarray(emb, np.float32).T).astype(BF16),
        "wxT": np.ascontiguousarray(np.asarray(Wx_w, np.float32).T).astype(BF16),
        "whT": np.ascontiguousarray(np.asarray(Wh_w, np.float32).T).astype(BF16),
        "woT": np.ascontiguousarray(np.asarray(Wo_w, np.float32).T).astype(BF16),
        "biasx": (np.asarray(Wx_b, np.float32)
                  + np.asarray(Wh_b, np.float32)).reshape(1, HID).astype(BF16),
        "biaso": np.asarray(Wo_b, np.float32).reshape(1, NUM_C).astype(BF16),
        "eye": np.eye(128, dtype=np.float32).astype(BF16),
        "ones": np.ones((1, 128), np.float32).astype(BF16),
    }
    if general_tau:
        inv = (1.0 / np.asarray(tau, np.float32)).astype(np.float32)
        a = (1.0 - inv).reshape(KC, 128).T            # [128, KC]
        b = inv.reshape(KC, 128).T
        common["taua"] = np.repeat(a, B_LOC, axis=1).astype(np.float32)
        common["taub"] = np.repeat(b, B_LOC, axis=1).astype(np.float32)

    idx_full = (np.asarray(q, np.int64) + NUM_C * np.asarray(r, np.int64))
    idx_full = idx_full.astype(np.int16)              # values < 2048
    in_maps = []
    for c in range(N_CORES):
        shard = idx_full[c * B_LOC:(c + 1) * B_LOC, :S]     # [B_LOC, S]
        idx_sb = np.ascontiguousarray(shard.T).reshape(-1)  # s-major tokens
        ntok = idx_sb.size
        wrap = idx_sb.reshape(ntok // 512, 32, 16).transpose(2, 0, 1)
        wrap = np.ascontiguousarray(wrap).reshape(16, ntok // 16)
        idxs = np.tile(wrap, (8, 1)).astype(np.int16)       # [128, ntok//16]
        in_maps.append({**common, "idxs": idxs})
    return in_maps, general_tau


_NC_CACHE = {}


def _get_nc(S, general_tau):
    key = (S, general_tau)
    if key not in _NC_CACHE:
        _NC_CACHE[key] = build_nc(S=S, general_tau=general_tau)
    return _NC_CACHE[key]


def run(trace=False, S=SEQ, **inputs):
    in_maps, general_tau = _host_prep(S=S, **inputs)
    nc = _get_nc(S, general_tau)
    last_err = None
    for _attempt in range(3):   # NRT exec errors are occasionally transient
        try:
            res = run_bass_kernel_spmd(nc, in_maps,
                                       core_ids=list(range(N_CORES)),
                                       trace=trace)
            break
        except Exception as e:  # noqa: BLE001
            last_err = e
    else:
        raise last_err
    y = np.concatenate([r["y"] for r in res.results], axis=0)
    return y.astype(np.float32), res


def kernel(**inputs) -> np.ndarray:
    y, _ = run(trace=False, **inputs)
    return y
